# revision 9
# baseline (speedup 1.0000x reference)
"""NetVLAD forward kernel for Trainium2 (Bass/Tile), data-parallel over 8 cores.

Math (per image n, NetVLAD with K=64 clusters, C=256 channels, P=1024 pixels):
    xhat = x / ||x||_2 (over C, per pixel)
    logits = conv_w @ xhat + conv_b          (K, P)
    a = softmax_K(logits)
    vlad[k, :] = sum_p a[k,p] * (xhat[:,p] - centroids[k,:])
    vlad = rownorm(vlad); out = vlad.flatten() / ||.||  (= /8 exactly)

Implementation notes:
  - batch 128 sharded 16 images/core; all params replicated.
  - x kept in natural (C, P) layout for the logits matmul; transposed on the
    PE (matmul vs identity) into (P, C) for the aggregation matmul.
  - everything through the PE in bf16 (fp32 matmul is 4 cyc/row, bf16 is 1).
  - per-pixel 1/||x|| folded into the softmax-weight matrix lhsT; |asum| comes
    from an extra (-n_p) rhs column so a single PSUM tile yields both agg and
    asum.
  - rsqrt built as exp(-0.5*ln(.)) so ACT stays on one LUT table set
    (Ln/Exp/Copy/Square share `natural_log_exp_and_others`; Sqrt does not).
  - softmax computed without max-subtraction: |logits| <= ~0.7 by
    Cauchy-Schwarz (||w_k|| <= 0.61, ||xhat||=1, |b| <= 1/16), so exp is safe.
"""

import sys

sys.path.insert(0, "/opt/trn_rl_repo")

import contextlib
import math

import numpy as np
import ml_dtypes

import concourse.bass as bass
import concourse.bacc as bacc
import concourse.tile as tile
from concourse import mybir
from concourse.bass_utils import run_bass_kernel_spmd

F32 = mybir.dt.float32
BF16 = mybir.dt.bfloat16
BF = ml_dtypes.bfloat16

N_CORES = 8
N_IMG = 16          # images per core
C = 256             # channels
K = 64              # clusters
P = 1024            # pixels (32*32)
CB = C // 128       # channel blocks
PB = P // 128       # pixel blocks
LN8 = math.log(8.0)

Alu = mybir.AluOpType
Act = mybir.ActivationFunctionType


def build_nc(n_img=N_IMG, repeat=1):
    nc = bacc.Bacc("TRN2", target_bir_lowering=False, debug=False,
                   num_devices=N_CORES)

    x_d = nc.dram_tensor("x", [n_img, C, P], F32, kind="ExternalInput").ap()
    wt_d = nc.dram_tensor("wt", [128, CB, K], BF16, kind="ExternalInput").ap()
    bbc_d = nc.dram_tensor("bbc", [128, PB, K], F32, kind="ExternalInput").ap()
    cent_d = nc.dram_tensor("cent", [K, C], F32, kind="ExternalInput").ap()
    id_d = nc.dram_tensor("ident", [128, 128], BF16, kind="ExternalInput").ap()
    ones_d = nc.dram_tensor("ones1", [128, 1], BF16, kind="ExternalInput").ap()
    y_d = nc.dram_tensor("y", [n_img, K * C], F32, kind="ExternalOutput").ap()

    with tile.TileContext(nc) as tc:
        with contextlib.ExitStack() as ctx:
            consts = ctx.enter_context(tc.tile_pool(name="consts", bufs=1))
            xpool = ctx.enter_context(tc.tile_pool(name="xpool", bufs=3))
            xbpool = ctx.enter_context(tc.tile_pool(name="xbpool", bufs=3))
            sqpool = ctx.enter_context(tc.tile_pool(name="sqpool", bufs=2))
            xtspool = ctx.enter_context(tc.tile_pool(name="xtspool", bufs=2))
            smpool = ctx.enter_context(tc.tile_pool(name="smpool", bufs=2))
            tiny = ctx.enter_context(tc.tile_pool(name="tiny", bufs=3))
            fpool = ctx.enter_context(tc.tile_pool(name="fpool", bufs=2))
            ps_n2 = ctx.enter_context(
                tc.tile_pool(name="ps_n2", bufs=1, space="PSUM"))
            ps_raw = ctx.enter_context(
                tc.tile_pool(name="ps_raw", bufs=2, space="PSUM"))
            ps_xt = ctx.enter_context(
                tc.tile_pool(name="ps_xt", bufs=4, space="PSUM"))
            ps_v = ctx.enter_context(
                tc.tile_pool(name="ps_v", bufs=1, space="PSUM"))

            wt_t = consts.tile([128, CB, K], BF16)
            nc.sync.dma_start(wt_t[:], wt_d[:])
            bbc_t = consts.tile([128, PB, K], F32)
            nc.sync.dma_start(bbc_t[:], bbc_d[:])
            cent_t = consts.tile([K, C], F32)
            nc.sync.dma_start(cent_t[:], cent_d[:])
            id_t = consts.tile([128, 128], BF16)
            nc.sync.dma_start(id_t[:], id_d[:])
            ones_t = consts.tile([128, 1], BF16)
            nc.sync.dma_start(ones_t[:], ones_d[:])
            ln8_t = consts.tile([128, 1], F32)
            nc.vector.memset(ln8_t[:], -LN8)

            for _rep in range(repeat):
                for n in range(n_img):
                    # ---- load x[n] as (128, CB, 1024) (partition = c % 128)
                    x_t = xpool.tile([128, CB, P], F32, tag="x")
                    nc.sync.dma_start(
                        x_t[:], x_d[n].rearrange("(cb p) m -> p cb m", p=128))

                    # ---- cast to bf16 (GPSIMD, otherwise idle)
                    xb_t = xbpool.tile([128, CB, P], BF16, tag="xb")
                    for cb in range(CB):
                        nc.gpsimd.tensor_copy(xb_t[:, cb, :], x_t[:, cb, :])

                    # ---- n2_p = sum_c x^2 : square on DVE, reduce on PE
                    sq_t = sqpool.tile([128, CB, P], BF16, tag="sq")
                    nc.vector.tensor_mul(
                        sq_t[:].rearrange("p a b -> p (a b)"),
                        xb_t[:].rearrange("p a b -> p (a b)"),
                        xb_t[:].rearrange("p a b -> p (a b)"))
                    n2_p = ps_n2.tile([128, PB], F32, tag="n2")
                    for pb in range(PB):
                        for cb in range(CB):
                            nc.tensor.matmul(
                                n2_p[:, pb:pb + 1],
                                sq_t[:, cb, pb * 128:(pb + 1) * 128],
                                ones_t[:],
                                start=(cb == 0), stop=(cb == CB - 1))

                    # ---- invn = n2^-0.5, ncol = -n2^0.5 (Ln/Exp based)
                    ln2_t = tiny.tile([128, PB], F32, tag="ln2")
                    nc.scalar.activation(ln2_t[:], n2_p[:], Act.Ln)
                    invn_t = tiny.tile([128, PB], F32, tag="invn")
                    nc.scalar.activation(invn_t[:], ln2_t[:], Act.Exp, scale=-0.5)
                    nn_t = tiny.tile([128, PB], F32, tag="nn")
                    nc.scalar.activation(nn_t[:], ln2_t[:], Act.Exp, scale=0.5)
                    ncb_t = tiny.tile([128, PB], BF16, tag="ncb")
                    nc.vector.tensor_scalar_mul(ncb_t[:], nn_t[:], -1.0)

                    # ---- logits (pre-scale) + transpose, fused over PE
                    raw_p = ps_raw.tile([128, PB * K], F32, tag="raw")
                    xts_t = xtspool.tile([128, PB, 257], BF16, tag="xts")
                    # -n column for the asum trick, via SBUF->SBUF DMA
                    # (strided narrow compute-engine writes are broken)
                    nc.sync.dma_start(xts_t[:, :, 256], ncb_t[:])
                    for g in range(4):
                        xt_g = ps_xt.tile([128, 2 * 256], F32, tag="xtp")
                        for pl in range(2):
                            pb = g * 2 + pl
                            for cb in range(CB):
                                blk = xb_t[:, cb, pb * 128:(pb + 1) * 128]
                                nc.tensor.matmul(
                                    raw_p[:, pb * K:(pb + 1) * K],
                                    blk, wt_t[:, cb, :],
                                    start=(cb == 0), stop=(cb == CB - 1))
                                nc.tensor.matmul(
                                    xt_g[:, pl * 256 + cb * 128:
                                         pl * 256 + (cb + 1) * 128],
                                    blk, id_t[:], start=True, stop=True)
                        # copy transposed chunk to SBUF bf16 (split ACT/DVE)
                        dst = xts_t[:, g * 2:(g + 1) * 2, 0:256]
                        if g % 2 == 0:
                            nc.scalar.copy(dst, xt_g[:].rearrange(
                                "p (a b) -> p a b", a=2))
                        else:
                            nc.vector.tensor_copy(dst, xt_g[:].rearrange(
                                "p (a b) -> p a b", a=2))

                    # ---- logits = raw * invn + b ; e = exp(logits)
                    l_t = smpool.tile([128, PB, K], F32, tag="l")
                    nc.vector.tensor_tensor(
                        l_t[:], raw_p[:].rearrange("p (a b) -> p a b", a=PB),
                        invn_t[:].broadcast_to((128, PB, K)), op=Alu.mult)
                    nc.vector.tensor_tensor(l_t[:], l_t[:], bbc_t[:], op=Alu.add)
                    e_t = smpool.tile([128, PB, K], F32, tag="e")
                    nc.scalar.activation(
                        e_t[:].rearrange("p a b -> p (a b)"),
                        l_t[:].rearrange("p a b -> p (a b)"), Act.Exp)

                    # ---- softmax scale s = invn / S ; aT = e * s
                    s_t = tiny.tile([128, PB], F32, tag="s")
                    nc.vector.tensor_reduce(
                        s_t[:], e_t[:], axis=mybir.AxisListType.X, op=Alu.add)
                    nc.vector.reciprocal(s_t[:], s_t[:])
                    nc.vector.tensor_mul(s_t[:], s_t[:], invn_t[:])
                    at_t = smpool.tile([128, PB, K], BF16, tag="at")
                    nc.vector.tensor_tensor(
                        at_t[:], e_t[:],
                        s_t[:].broadcast_to((128, PB, K)), op=Alu.mult)

                    # ---- aggregation: psum_v = [agg | -asum]
                    v_p = ps_v.tile([K, C + 1], F32, tag="v")
                    for pb in range(PB):
                        nc.tensor.matmul(
                            v_p[:], at_t[:, pb, :], xts_t[:, pb, :],
                            start=(pb == 0), stop=(pb == PB - 1))

                    # ---- vlad = agg - asum*cent ; normalize rows ; /8
                    as_t = fpool.tile([K, 1], F32, tag="as")
                    nc.scalar.copy(as_t[:], v_p[:, C:C + 1])
                    w_t = fpool.tile([K, C], F32, tag="w")
                    nc.vector.scalar_tensor_tensor(
                        w_t[:], cent_t[:], as_t[:], v_p[:, 0:C],
                        op0=Alu.mult, op1=Alu.add)
                    wsq_t = fpool.tile([K, C], F32, tag="wsq")
                    nc.vector.tensor_mul(wsq_t[:], w_t[:], w_t[:])
                    n2v_t = fpool.tile([K, 1], F32, tag="n2v")
                    nc.vector.tensor_reduce(
                        n2v_t[:], wsq_t[:], axis=mybir.AxisListType.X,
                        op=Alu.add)
                    lnv_t = fpool.tile([K, 1], F32, tag="lnv")
                    nc.scalar.activation(lnv_t[:], n2v_t[:], Act.Ln)
                    f_t = fpool.tile([K, 1], F32, tag="f")
                    nc.scalar.activation(f_t[:], lnv_t[:], Act.Exp,
                                         scale=-0.5, bias=ln8_t[0:K, :])
                    out_t = fpool.tile([K, C], F32, tag="out")
                    nc.vector.tensor_scalar_mul(out_t[:], w_t[:], f_t[:])

                    nc.sync.dma_start(
                        y_d[n].rearrange("(k c) -> k c", k=K), out_t[:])

    nc.compile()
    return nc


def make_const_inputs(conv_w, conv_b, centroids):
    wt = np.ascontiguousarray(
        conv_w.T.reshape(CB, 128, K).transpose(1, 0, 2)).astype(BF)
    bbc = np.ascontiguousarray(
        np.broadcast_to(conv_b.astype(np.float32), (128, PB, K)))
    cent = np.ascontiguousarray(centroids.astype(np.float32))
    ident = np.eye(128, dtype=BF)
    ones1 = np.ones((128, 1), dtype=BF)
    return {"wt": wt, "bbc": bbc, "cent": cent, "ident": ident, "ones1": ones1}


_NC_CACHE = {}


def _get_nc(n_img=N_IMG, repeat=1):
    key = (n_img, repeat)
    if key not in _NC_CACHE:
        _NC_CACHE[key] = build_nc(n_img, repeat)
    return _NC_CACHE[key]


def run_on_cores(nc, x, consts):
    n_img_core = x.shape[0] // N_CORES
    xr = np.ascontiguousarray(
        x.reshape(N_CORES, n_img_core, C, P).astype(np.float32))
    in_maps = [{"x": xr[i], **consts} for i in range(N_CORES)]
    res = run_bass_kernel_spmd(nc, in_maps, list(range(N_CORES)))
    return np.concatenate([r["y"] for r in res.results], axis=0)


def kernel(x, conv_w, conv_b, centroids):
    x = np.asarray(x)
    N = x.shape[0]
    nc = _get_nc(N // N_CORES)
    consts = make_const_inputs(np.asarray(conv_w), np.asarray(conv_b),
                               np.asarray(centroids))
    y = run_on_cores(nc, x.reshape(N, C, P), consts)
    return y.reshape(N, K * C).astype(np.float32)
